# revision 1
# baseline (speedup 1.0000x reference)
"""ChatGLM3 attention (B=2, S=2048, H=4096, 32 q-heads / 2 kv-heads, D=128)
on 8 Trainium2 NeuronCores.

Sharding: core c = 4*b + tp  (b in {0,1} data-parallel over batch,
tp in {0..3} tensor-parallel over heads). Each core computes the QKV
projection for its 8 q-heads + its kv head (k and v columns), applies RoPE,
and runs causal GQA attention for its 8 heads over the full sequence.
No collectives; per-core inputs/outputs are sharded and assembled on host.

Device layout is "transposed": qkvT [n, s] with head-dim on partitions, so
the projection needs no transposes (lhsT = w columns, rhs = hiddenT) and
scores come out as scoresT [k, s_q] whose softmax sum is done with an
ones-vector matmul. All matmuls run in float32r (TF32-like, full fp32
storage, ~2e-4 matmul error).
"""
import numpy as np
from contextlib import ExitStack

import concourse.bacc as bacc
import concourse.tile as tile
import concourse.mybir as mybir

# Problem constants (hardcoded per contract)
B, S, HIDDEN = 2, 2048, 4096
NUM_HEADS, NUM_KV_HEADS, D = 32, 2, 128
ROPE_BASE = 10000.0
N_CORES = 8
HEADS_PER_CORE = NUM_HEADS // 4          # 8 (TP=4)
NC_CHUNKS = HEADS_PER_CORE + 2           # 8 q + 1 k + 1 v = 10 n-chunks of 128
SC = 512                                 # s-chunk (psum bank = 512 fp32)
NSC = S // SC                            # 4
NKT = S // 128                           # 16 k-tiles
SCALE = float(D) ** -0.5
BIG = 30000.0

f32 = mybir.dt.float32
f32r = mybir.dt.float32r

_CACHE: dict = {}


def _build_nc(loop_n: int = 1, parts=("proj", "attn")):
    nc = bacc.Bacc(trn_type="TRN2", target_bir_lowering=False, debug=False)

    hT_d = nc.dram_tensor("hT", [HIDDEN, S], f32, kind="ExternalInput").ap()
    # wc pre-tiled on host: wc_t[n*128+p, k*128+m] = w_slice[k*128+p, n*128+m]
    wc_d = nc.dram_tensor("wc", [NC_CHUNKS * 128, HIDDEN], f32, kind="ExternalInput").ap()
    cosF_d = nc.dram_tensor("cosF", [128, S], f32, kind="ExternalInput").ap()
    sinS_d = nc.dram_tensor("sinS", [128, S], f32, kind="ExternalInput").ap()
    ident_d = nc.dram_tensor("ident", [128, 128], f32, kind="ExternalInput").ap()
    perm_d = nc.dram_tensor("perm", [128, 128], f32, kind="ExternalInput").ap()
    trione_d = nc.dram_tensor("trione", [128, 128], f32, kind="ExternalInput").ap()
    maskB_d = nc.dram_tensor("maskB", [128, 4 * SC], f32, kind="ExternalInput").ap()
    onesc_d = nc.dram_tensor("onesc", [128, 1], f32, kind="ExternalInput").ap()
    onesr_d = nc.dram_tensor("onesr", [1, 128], f32, kind="ExternalInput").ap()
    outT_d = nc.dram_tensor("outT", [HEADS_PER_CORE * 128, S], f32, kind="ExternalOutput").ap()

    with tile.TileContext(nc) as tc, ExitStack() as ctx:
        if loop_n > 1:
            ctx.enter_context(tc.For_i(0, loop_n, 1))
        cpool = ctx.enter_context(tc.tile_pool(name="consts", bufs=1))
        qk_pool = ctx.enter_context(tc.tile_pool(name="qkt", bufs=1))
        v_pool = ctx.enter_context(tc.tile_pool(name="vsb", bufs=1))

        ident = cpool.tile([128, 128], f32r, tag="ident")
        perm = cpool.tile([128, 128], f32r, tag="perm")
        trione = cpool.tile([128, 128], f32r, tag="trione")
        maskB = cpool.tile([128, 4 * SC], f32r, tag="maskB")
        ones_c = cpool.tile([128, 1], f32r, tag="onesc")
        ones_r = cpool.tile([1, 128], f32r, tag="onesr")
        nc.sync.dma_start(ident[:], ident_d.bitcast(f32r))
        nc.sync.dma_start(perm[:], perm_d.bitcast(f32r))
        nc.sync.dma_start(trione[:], trione_d.bitcast(f32r))
        nc.sync.dma_start(maskB[:], maskB_d.bitcast(f32r))
        nc.sync.dma_start(ones_c[:], onesc_d.bitcast(f32r))
        nc.sync.dma_start(ones_r[:], onesr_d.bitcast(f32r))

        # persistent: 8 q heads + k, all RoPE'd, [d, s] layout
        qkT = [
            qk_pool.tile([128, S], f32r, tag=f"qkT{n}", name=f"qkT{n}")
            for n in range(9)
        ]
        # v in [s, d] layout: tile t at columns t*128:(t+1)*128
        v_sb = v_pool.tile([128, NKT * 128], f32r, tag="vsb")

        # shared pools — proj(sc+1) DMA/PE overlaps attention(qc=sc)
        hts_pool = ctx.enter_context(tc.tile_pool(name="hts", bufs=32))
        w_pool = ctx.enter_context(tc.tile_pool(name="wt", bufs=3))
        tab_pool = ctx.enter_context(tc.tile_pool(name="tabs", bufs=1))
        rope_pool = ctx.enter_context(tc.tile_pool(name="rope", bufs=2))
        probs_pool = ctx.enter_context(tc.tile_pool(name="probs", bufs=3))
        att_pool = ctx.enter_context(tc.tile_pool(name="att", bufs=1))
        pp = ctx.enter_context(tc.tile_pool(name="pp", bufs=1, space="PSUM"))
        ptmp = ctx.enter_context(tc.tile_pool(name="ptmp", bufs=2, space="PSUM"))
        scp = ctx.enter_context(tc.tile_pool(name="scp", bufs=3, space="PSUM"))
        pvl = ctx.enter_context(tc.tile_pool(name="pvl", bufs=1, space="PSUM"))

        kT = qkT[8]
        HK = HIDDEN // 2
        for sc in range(NSC):
            ssl = slice(sc * SC, (sc + 1) * SC)
            # ---- projection pass for this s-chunk ----
            ht = []
            for k in range(HIDDEN // 128):
                t = hts_pool.tile([128, SC], f32r, tag="ht")
                nc.sync.dma_start(
                    t[:], hT_d[k * 128:(k + 1) * 128, ssl].bitcast(f32r))
                ht.append(t)
            cos_t = tab_pool.tile([128, SC], f32, tag="cos")
            sin_t = tab_pool.tile([128, SC], f32, tag="sin")
            nc.sync.dma_start(cos_t[:], cosF_d[:, ssl])
            nc.sync.dma_start(sin_t[:], sinS_d[:, ssl])

            for n in range(NC_CHUNKS):
                wnA = w_pool.tile([128, HK], f32r, tag="wn", name="wnA")
                wnB = w_pool.tile([128, HK], f32r, tag="wn", name="wnB")
                nc.sync.dma_start(
                    wnA[:], wc_d[n * 128:(n + 1) * 128, :HK].bitcast(f32r))
                nc.sync.dma_start(
                    wnB[:], wc_d[n * 128:(n + 1) * 128, HK:].bitcast(f32r))
                psum = pp.tile([128, SC], f32, tag="proj")
                for k in range(HIDDEN // 128):
                    w_src = wnA if k < 16 else wnB
                    kk = k % 16
                    nc.tensor.matmul(
                        psum[:], w_src[:, kk * 128:(kk + 1) * 128], ht[k][:],
                        start=(k == 0), stop=(k == HIDDEN // 128 - 1),
                    )
                if n < 9:
                    # RoPE: out = raw*cos + swap(raw)*sin_signed
                    qraw = rope_pool.tile([128, SC], f32r, tag="qraw")
                    nc.scalar.copy(qraw[:], psum[:])
                    swps = ptmp.tile([128, SC], f32, tag="ptmp", name="swps")
                    nc.tensor.matmul(swps[:], perm[:], qraw[:], start=True, stop=True)
                    t1 = rope_pool.tile([128, SC], f32, tag="t1")
                    nc.vector.tensor_mul(t1[:], qraw[:].bitcast(f32), cos_t[:])
                    t2 = rope_pool.tile([128, SC], f32, tag="t2")
                    nc.vector.tensor_mul(t2[:], swps[:], sin_t[:])
                    with nc.allow_low_precision(reason="fp32r rounding of rope output"):
                        nc.vector.tensor_add(qkT[n][:, ssl], t1[:], t2[:])
                else:
                    vraw = rope_pool.tile([128, SC], f32r, tag="qraw")
                    nc.scalar.copy(vraw[:], psum[:])
                    for j in range(SC // 128):
                        vt = ptmp.tile([128, 128], f32r, tag="ptmp", name="vt")
                        with nc.allow_low_precision(reason="fp32r transpose"):
                            nc.tensor.transpose(
                                vt[:], vraw[:, j * 128:(j + 1) * 128], ident[:]
                            )
                        kt_glob = sc * (SC // 128) + j
                        nc.vector.tensor_copy(
                            v_sb[:, kt_glob * 128:(kt_glob + 1) * 128], vt[:]
                        )

            # ---- attention for q-chunk qc == sc (all heads) ----
            qc = sc
            n_kt = (qc + 1) * (SC // 128)
            for h in range(HEADS_PER_CORE):
                pv = pvl.tile([128, SC], f32, tag="pv")
                lacc = pvl.tile([1, SC], f32, tag="lacc")
                for kt in range(n_kt):
                    oo = kt * 128 - qc * SC
                    is_partial = 0 <= oo < SC
                    scps = scp.tile([128, SC], f32, tag="sc")
                    nc.tensor.matmul(
                        scps[:], kT[:, kt * 128:(kt + 1) * 128],
                        qkT[h][:, qc * SC:(qc + 1) * SC],
                        start=True, stop=not is_partial,
                    )
                    if is_partial:
                        nc.tensor.matmul(
                            scps[:], trione[:],
                            maskB[:, (oo // 128) * SC:(oo // 128 + 1) * SC],
                            start=False, stop=True,
                        )
                    probs = probs_pool.tile([128, SC], f32r, tag="probs")
                    nc.scalar.activation(
                        probs[:], scps[:], mybir.ActivationFunctionType.Exp,
                        scale=SCALE,
                    )
                    nc.tensor.matmul(
                        pv[:], v_sb[:, kt * 128:(kt + 1) * 128], probs[:],
                        start=(kt == 0), stop=(kt == n_kt - 1),
                    )
                    nc.tensor.matmul(
                        lacc[:], ones_c[:], probs[:],
                        start=(kt == 0), stop=(kt == n_kt - 1),
                    )
                lrec = att_pool.tile([1, SC], f32r, tag="lrec")
                with nc.allow_low_precision(reason="softmax denom recip"):
                    nc.vector.reciprocal(lrec[:], lacc[:])
                lexp_ps = ptmp.tile([128, SC], f32, tag="ptmp", name="lexp_ps")
                nc.tensor.matmul(lexp_ps[:], ones_r[:], lrec[:], start=True, stop=True)
                lexp = att_pool.tile([128, SC], f32, tag="lexpsb")
                nc.scalar.copy(lexp[:], lexp_ps[:])
                outn = att_pool.tile([128, SC], f32, tag="outn")
                nc.vector.tensor_mul(outn[:], pv[:], lexp[:])
                nc.sync.dma_start(
                    outT_d[h * 128:(h + 1) * 128, qc * SC:(qc + 1) * SC], outn[:]
                )

    nc.finalize()
    return nc


def _get_runner(loop_n: int = 1):
    """Build nc once and a cached jitted shard_map callable (axon/PJRT)."""
    key = f"runner{loop_n}"
    if key in _CACHE:
        return _CACHE[key]

    import jax
    import jax.numpy as jnp  # noqa: F401
    from jax.sharding import Mesh, PartitionSpec
    from jax.experimental.shard_map import shard_map
    from concourse.bass2jax import (
        install_neuronx_cc_hook, _bass_exec_p, partition_id_tensor,
    )
    import concourse.mybir as _mybir

    nc = _build_nc(loop_n)
    install_neuronx_cc_hook()

    partition_name = nc.partition_id_tensor.name if nc.partition_id_tensor else None
    in_names, out_names, out_avals, zero_outs = [], [], [], []
    for alloc in nc.m.functions[0].allocations:
        if not isinstance(alloc, _mybir.MemoryLocationSet):
            continue
        name = alloc.memorylocations[0].name
        if alloc.kind == "ExternalInput":
            if name != partition_name:
                in_names.append(name)
        elif alloc.kind == "ExternalOutput":
            shape = tuple(alloc.tensor_shape)
            npdt = _mybir.dt.np(alloc.dtype)
            out_avals.append(jax.core.ShapedArray(shape, npdt))
            out_names.append(name)
            zero_outs.append(np.zeros(shape, npdt))

    n_params = len(in_names)
    n_outs = len(out_avals)
    all_in_names = in_names + out_names
    if partition_name is not None:
        all_in_names.append(partition_name)
    donate = tuple(range(n_params, n_params + n_outs))

    def _body(*args):
        operands = list(args)
        if partition_name is not None:
            operands.append(partition_id_tensor())
        outs = _bass_exec_p.bind(
            *operands,
            out_avals=tuple(out_avals),
            in_names=tuple(all_in_names),
            out_names=tuple(out_names),
            lowering_input_output_aliases=(),
            sim_require_finite=True,
            sim_require_nnan=True,
            nc=nc,
        )
        return tuple(outs)

    devices = jax.devices()[:N_CORES]
    mesh = Mesh(np.asarray(devices), ("core",))
    in_specs = (PartitionSpec("core"),) * (n_params + n_outs)
    out_specs = (PartitionSpec("core"),) * n_outs
    fn = jax.jit(
        shard_map(_body, mesh=mesh, in_specs=in_specs, out_specs=out_specs,
                  check_rep=False),
        donate_argnums=donate,
        keep_unused=True,
    )

    runner = (fn, in_names, out_names, out_avals, zero_outs)
    _CACHE[key] = runner
    return runner


def _host_prep(positions, hidden_states, w_qkv):
    """Build the per-core input maps (shard + layout prep, no reference math)."""
    positions = np.asarray(positions)
    hidden_states = np.ascontiguousarray(np.asarray(hidden_states, dtype=np.float32))
    w_qkv = np.ascontiguousarray(np.asarray(w_qkv, dtype=np.float32))

    half = D // 2
    inv_freq = 1.0 / (ROPE_BASE ** (np.arange(half, dtype=np.float32) / half))
    ang = positions.astype(np.float32)[:, None] * inv_freq[None, :]  # [S, 64]
    cos = np.cos(ang).astype(np.float32)  # [S, 64]
    sin = np.sin(ang).astype(np.float32)
    cosF = np.empty((128, S), np.float32)
    sinS = np.empty((128, S), np.float32)
    cosF[:half] = cos.T
    cosF[half:] = cos.T
    sinS[:half] = -sin.T
    sinS[half:] = sin.T

    ident = np.eye(128, dtype=np.float32)
    perm = np.roll(np.eye(128, dtype=np.float32), 64, axis=0)
    trione = np.triu(np.ones((128, 128), np.float32))
    maskB = np.zeros((4, 128, SC), np.float32)
    for oi, o in enumerate([0, 128, 256, 384]):
        for c in range(1, 128):
            qq = c + o - 1
            if 0 <= qq < SC:
                maskB[oi, c, qq] = -BIG
        maskB[oi, 0, :o] = -BIG
    maskB_flat = np.ascontiguousarray(
        maskB.transpose(1, 0, 2).reshape(128, 4 * SC)
    )
    onesc = np.ones((128, 1), np.float32)
    onesr = np.ones((1, 128), np.float32)

    hT = [np.ascontiguousarray(hidden_states[b].T) for b in range(B)]

    q_sz = NUM_HEADS * D
    in_maps = []
    for c in range(N_CORES):
        b, tp = divmod(c, 4)
        kv = tp // 2
        wq = w_qkv[:, tp * 1024:(tp + 1) * 1024]
        wk = w_qkv[:, q_sz + kv * 128: q_sz + (kv + 1) * 128]
        wv = w_qkv[:, q_sz + NUM_KV_HEADS * D + kv * 128:
                      q_sz + NUM_KV_HEADS * D + (kv + 1) * 128]
        wc = np.concatenate([wq, wk, wv], axis=1)  # [4096, 1280]
        # tile to [n*128+p, k*128+m] = wc[k*128+p, n*128+m]
        wc_t = np.ascontiguousarray(
            wc.reshape(HIDDEN // 128, 128, NC_CHUNKS, 128)
            .transpose(2, 1, 0, 3)
            .reshape(NC_CHUNKS * 128, HIDDEN)
        )
        in_maps.append({
            "hT": hT[b], "wc": wc_t, "cosF": cosF, "sinS": sinS,
            "ident": ident, "perm": perm, "trione": trione,
            "maskB": maskB_flat, "onesc": onesc, "onesr": onesr,
        })
    return in_maps


def run_device(in_maps):
    """Run the compiled kernel on 8 cores; returns list of per-core outputs."""
    fn, in_names, out_names, out_avals, zero_outs = _get_runner()
    per_core = [[np.asarray(m[nm]) for nm in in_names] for m in in_maps]
    concat_in = [
        np.concatenate([per_core[c][i] for c in range(N_CORES)], axis=0)
        for i in range(len(in_names))
    ]
    concat_zeros = [
        np.zeros((N_CORES * z.shape[0], *z.shape[1:]), z.dtype) for z in zero_outs
    ]
    out_arrs = fn(*concat_in, *concat_zeros)
    return [
        {
            nm: np.asarray(out_arrs[i]).reshape(N_CORES, *out_avals[i].shape)[c]
            for i, nm in enumerate(out_names)
        }
        for c in range(N_CORES)
    ]


def kernel(positions, hidden_states, w_qkv):
    in_maps = _host_prep(positions, hidden_states, w_qkv)
    results = run_device(in_maps)
    out = np.empty((B, S, NUM_HEADS * D), np.float32)
    for c in range(N_CORES):
        b, tp = divmod(c, 4)
        oT = results[c]["outT"].reshape(HEADS_PER_CORE, 128, S)
        out[b, :, tp * 1024:(tp + 1) * 1024] = (
            oT.transpose(2, 0, 1).reshape(S, HEADS_PER_CORE * 128)
        )
    return out



# revision 33
# speedup vs baseline: 1.0612x; 1.0612x over previous
"""ChatGLM3 attention (B=2, S=2048, H=4096, 32 q-heads / 2 kv-heads, D=128)
on 8 Trainium2 NeuronCores.

Sharding: core c = 4*b + tp  (b in {0,1} data-parallel over batch,
tp in {0..3} tensor-parallel over heads). Each core computes the QKV
projection for its 8 q-heads + its kv head (k and v columns), applies RoPE,
and runs causal GQA attention for its 8 heads over the full sequence.
No collectives; per-core inputs/outputs are sharded and assembled on host.

Device layout is "transposed": qkvT [n, s] with head-dim on partitions, so
the projection needs no transposes (lhsT = w columns, rhs = hiddenT) and
scores come out as scoresT [k, s_q] whose softmax sum is done with an
ones-vector matmul. Projection operands (weights, hidden) are bf16 to halve
DMA traffic; the whole data path runs in bf16 with fp32 PSUM accumulation so
every matmul's stationary operand loads via a separately-schedulable
(hideable) Ldweights instruction.

Pipelining: attention is kt-outer / 3-head-group-inner so the kT/v
stationary operands are loaded once per k-tile instead of once per head,
RoPE and the pv/lacc matmuls lag their producers by one step so the PE
never waits on the Activation/Vector engines, and the Activation engine
runs exclusively Exp (no activation-table switches).
"""
import numpy as np
from contextlib import ExitStack

import concourse.bacc as bacc
import concourse.tile as tile
import concourse.mybir as mybir

# Problem constants (hardcoded per contract)
B, S, HIDDEN = 2, 2048, 4096
NUM_HEADS, NUM_KV_HEADS, D = 32, 2, 128
ROPE_BASE = 10000.0
N_CORES = 8
HEADS_PER_CORE = NUM_HEADS // 4          # 8 (TP=4)
NC_CHUNKS = HEADS_PER_CORE + 2           # 8 q + 1 k + 1 v = 10 n-chunks of 128
SC = 512                                 # s-chunk (psum bank = 512 fp32)
NSC = S // SC                            # 4
NKT = S // 128                           # 16 k-tiles
SCALE = float(D) ** -0.5
BIG = 30000.0

f32 = mybir.dt.float32
f32r = mybir.dt.float32r
bf16 = mybir.dt.bfloat16

_CACHE: dict = {}


def _build_nc(loop_n: int = 1):
    nc = bacc.Bacc(trn_type="TRN2", target_bir_lowering=False, debug=False)

    hT_d = nc.dram_tensor("hT", [HIDDEN, S], bf16, kind="ExternalInput").ap()
    # wc pre-tiled on host: wc_t[n*128+p, k*128+m] = w_slice[k*128+p, n*128+m]
    wc_d = nc.dram_tensor("wc", [NC_CHUNKS * 128, HIDDEN], bf16, kind="ExternalInput").ap()
    cosF_d = nc.dram_tensor("cosF", [128, S], bf16, kind="ExternalInput").ap()
    sinS_d = nc.dram_tensor("sinS", [128, S], bf16, kind="ExternalInput").ap()
    ident_d = nc.dram_tensor("ident", [128, 128], bf16, kind="ExternalInput").ap()
    perm_d = nc.dram_tensor("perm", [128, 128], bf16, kind="ExternalInput").ap()
    mask01_d = nc.dram_tensor("mask01", [128, 4 * SC], bf16, kind="ExternalInput").ap()
    onesc_d = nc.dram_tensor("onesc", [128, 1], bf16, kind="ExternalInput").ap()
    onesr_d = nc.dram_tensor("onesr", [128, 128], bf16, kind="ExternalInput").ap()
    outT_d = nc.dram_tensor("outT", [HEADS_PER_CORE * 128, S], f32, kind="ExternalOutput").ap()

    # attention head-groups sized by PSUM banks: 3 pv + 3 lacc + 2 spp = 8
    GROUPS = [[0, 1, 2], [3, 4, 5], [6, 7]]

    with tile.TileContext(nc) as tc, ExitStack() as ctx:
        if loop_n > 1:
            ctx.enter_context(tc.For_i(0, loop_n, 1))
        cpool = ctx.enter_context(tc.tile_pool(name="consts", bufs=1))
        qk_pool = ctx.enter_context(tc.tile_pool(name="qkt", bufs=1))
        v_pool = ctx.enter_context(tc.tile_pool(name="vsb", bufs=1))

        ident = cpool.tile([128, 128], bf16, tag="ident")
        perm = cpool.tile([128, 128], bf16, tag="perm")
        mask01 = cpool.tile([128, 4 * SC], bf16, tag="mask01")
        ones_c = cpool.tile([128, 1], bf16, tag="onesc")
        ones_r = cpool.tile([128, 128], bf16, tag="onesr")
        nc.sync.dma_start(ident[:], ident_d)
        nc.sync.dma_start(perm[:], perm_d)
        nc.sync.dma_start(mask01[:], mask01_d)
        nc.sync.dma_start(ones_c[:], onesc_d)
        nc.sync.dma_start(ones_r[:], onesr_d)

        # persistent: 8 q heads + k, all RoPE'd, [d, s] layout
        qkT = [
            qk_pool.tile([128, S], bf16, tag=f"qkT{n}", name=f"qkT{n}")
            for n in range(9)
        ]
        # v in [s, d] layout: tile t at columns t*128:(t+1)*128
        v_sb = v_pool.tile([128, NKT * 128], bf16, tag="vsb")

        hts_pool = ctx.enter_context(tc.tile_pool(name="hts", bufs=40))
        w_pool = ctx.enter_context(tc.tile_pool(name="wt", bufs=3))
        tab_pool = ctx.enter_context(tc.tile_pool(name="tabs", bufs=2))
        rope_pool = ctx.enter_context(tc.tile_pool(name="rope", bufs=2))
        probs_pool = ctx.enter_context(tc.tile_pool(name="probs", bufs=10))
        att_pool = ctx.enter_context(tc.tile_pool(name="att", bufs=2))
        # PSUM: big (proj chains + pv accumulators) 4 banks, spp (rope swaps /
        # v transposes / score tiles) 2 banks, nrm (lacc rows + lexp bcast) 2.
        big = ctx.enter_context(tc.tile_pool(name="big", bufs=3, space="PSUM"))
        spp = ctx.enter_context(tc.tile_pool(name="spp", bufs=2, space="PSUM"))
        nrm = ctx.enter_context(tc.tile_pool(name="nrm", bufs=3, space="PSUM"))

        kT = qkT[8]

        # Deferred group normalization: emitted only after the next group's
        # (or next s-chunk's) PE stream has started, so the PE never idles
        # while the DVE drains reciprocal/rescale chains.
        pending_norm = [None]

        def flush_norm():
            if pending_norm[0] is not None:
                fn, pending_norm[0] = pending_norm[0], None
                fn()

        SH = 2 * SC                      # s-half width (two s-chunks)
        for sh in range(2):
            hsl = slice(sh * SH, (sh + 1) * SH)
            scs = [2 * sh, 2 * sh + 1]
            # ---- projection for this s-half (weights-outer: each weight
            # slice is loaded once and streams both s-chunks) ----
            ht = []
            for k in range(HIDDEN // 128):
                t = hts_pool.tile([128, SH], bf16, tag="ht")
                nc.sync.dma_start(t[:], hT_d[k * 128:(k + 1) * 128, hsl])
                ht.append(t)
            cos_t = tab_pool.tile([128, SH], bf16, tag="cos")
            sin_t = tab_pool.tile([128, SH], bf16, tag="sin")
            nc.sync.dma_start(cos_t[:], cosF_d[:, hsl])
            nc.sync.dma_start(sin_t[:], sinS_d[:, hsl])

            raws = {}

            def emit_rope(n):
                # RoPE for chunk n (both s-chunks of the half)
                for j in range(2):
                    qraw = raws[n][j]
                    jsl = slice(j * SC, (j + 1) * SC)
                    osl = slice((2 * sh + j) * SC, (2 * sh + j + 1) * SC)
                    swps = spp.tile([128, SC], f32, tag="spp", name="swps")
                    nc.tensor.matmul(swps[:], perm[:], qraw[:],
                                     start=True, stop=True)
                    with nc.allow_low_precision(reason="bf16 rope"):
                        swsb = rope_pool.tile([128, SC], bf16, tag="swsb")
                        nc.vector.tensor_copy(swsb[:], swps[:])
                        t1 = rope_pool.tile([128, SC], bf16, tag="t1")
                        nc.vector.tensor_mul(t1[:], qraw[:], cos_t[:, jsl])
                        t2 = rope_pool.tile([128, SC], bf16, tag="t2")
                        nc.vector.tensor_mul(t2[:], swsb[:], sin_t[:, jsl])
                        nc.vector.tensor_add(qkT[n][:, osl], t1[:], t2[:])

            HK = HIDDEN // 2
            # k-chunk (n=8) first so kT's RoPE is long done before the
            # attention phase needs it; v (n=9) last.
            order = [8] + list(range(8)) + [9]
            for idx, n in enumerate(order):
                wnA = w_pool.tile([128, HK], bf16, tag="wn", name="wnA")
                wnB = w_pool.tile([128, HK], bf16, tag="wn", name="wnB")
                nc.sync.dma_start(wnA[:], wc_d[n * 128:(n + 1) * 128, :HK])
                nc.sync.dma_start(wnB[:], wc_d[n * 128:(n + 1) * 128, HK:])
                p2 = [big.tile([128, SC], f32, tag="big", name=f"proj{j}")
                      for j in range(2)]
                for k in range(HIDDEN // 128):
                    w_src = wnA if k < 16 else wnB
                    kk = k % 16
                    for j in range(2):
                        nc.tensor.matmul(
                            p2[j][:], w_src[:, kk * 128:(kk + 1) * 128],
                            ht[k][:, j * SC:(j + 1) * SC],
                            start=(k == 0), stop=(k == HIDDEN // 128 - 1),
                        )
                if idx == 1:
                    # previous half's last attention group normalizes while
                    # these early projection chains stream
                    flush_norm()
                # PSUM eviction on DVE (Activation stays Exp-only)
                rr = []
                for j in range(2):
                    raw = rope_pool.tile([128, SC], bf16, tag="qraw", bufs=4,
                                         name="raw")
                    with nc.allow_low_precision(reason="bf16 qkv"):
                        nc.vector.tensor_copy(raw[:], p2[j][:])
                    rr.append(raw)
                raws[n] = rr
                # lagged RoPE: reads the previous chunk's raw, which DVE
                # finished copying while chain n ran.
                if idx >= 1:
                    emit_rope(order[idx - 1])

            def emit_vts(sc):
                # v transposes: [d, s-chunk] -> v_sb [s, d] blocks
                vraw = raws[9][sc - 2 * sh]
                for j in range(SC // 128):
                    vt = spp.tile([128, 128], bf16, tag="spp", name="vt")
                    with nc.allow_low_precision(reason="bf16 transpose"):
                        nc.tensor.transpose(
                            vt[:], vraw[:, j * 128:(j + 1) * 128], ident[:]
                        )
                    kt_glob = sc * (SC // 128) + j
                    nc.vector.tensor_copy(
                        v_sb[:, kt_glob * 128:(kt_glob + 1) * 128], vt[:]
                    )

            # ---- attention for the half's two q-chunks ----
            for qc in scs:
                qsl = slice(qc * SC, (qc + 1) * SC)
                n_kt = (qc + 1) * (SC // 128)
                for gi, heads in enumerate(GROUPS):
                    HG = len(heads)
                    pv = [big.tile([128, SC], f32, tag="big", name=f"pv{h}")
                          for h in heads]
                    laccs = [nrm.tile([128, SC], f32, tag="nrm",
                                      name=f"lacc{h}") for h in heads]
                    probs = {}

                    def emit_pv(kt):
                        last = kt == n_kt - 1
                        for i in range(HG):
                            nc.tensor.matmul(
                                pv[i][:], v_sb[:, kt * 128:(kt + 1) * 128],
                                probs[(kt, i)][:],
                                start=(kt == 0), stop=last,
                            )
                        for i in range(HG):
                            nc.tensor.matmul(
                                laccs[i][0:1, :], ones_c[:],
                                probs[(kt, i)][:],
                                start=(kt == 0), stop=last,
                            )

                    for kt in range(n_kt):
                        oo = kt * 128 - qc * SC
                        is_partial = 0 <= oo < SC
                        for i, h in enumerate(heads):
                            scps = spp.tile([128, SC], f32, tag="spp",
                                            name="sc")
                            nc.tensor.matmul(
                                scps[:], kT[:, kt * 128:(kt + 1) * 128],
                                qkT[h][:, qsl],
                                start=True, stop=True,
                            )
                            p = probs_pool.tile([128, SC], bf16, tag="probs")
                            nc.scalar.activation(
                                p[:], scps[:],
                                mybir.ActivationFunctionType.Exp,
                                scale=SCALE,
                            )
                            if is_partial:
                                # causal mask: zero invalid probs (bf16
                                # SBUF-only DVE op runs in 4x perf mode)
                                oi = oo // 128
                                nc.vector.tensor_mul(
                                    p[:], p[:],
                                    mask01[:, oi * SC:(oi + 1) * SC],
                                )
                            probs[(kt, i)] = p
                        if kt == 0:
                            # previous group's normalization overlaps this
                            # group's first score/exp wave
                            flush_norm()
                            if gi == 0:
                                emit_vts(qc)
                        # lagged pv/lacc: consume the previous kt's probs so
                        # the PE never waits on the Activation engine.
                        if kt >= 1:
                            emit_pv(kt - 1)
                            for i in range(HG):
                                del probs[(kt - 1, i)]
                    emit_pv(n_kt - 1)

                    def norm_group(pv=pv, laccs=laccs, heads=heads, qsl=qsl):
                        # all reciprocals first so the DVE has them ready
                        # before the PE reaches the lexp broadcasts
                        lrecs = []
                        for i in range(len(heads)):
                            lrec = att_pool.tile([1, SC], bf16, tag="lrec",
                                                 bufs=3, name="lrec")
                            with nc.allow_low_precision(reason="softmax recip"):
                                nc.vector.reciprocal(lrec[:], laccs[i][0:1, :])
                            lrecs.append(lrec)
                        for i, h in enumerate(heads):
                            lexp_ps = spp.tile([128, SC], f32, tag="spp",
                                               name="lexp_ps")
                            nc.tensor.matmul(lexp_ps[:], ones_r[0:1, :],
                                             lrecs[i][:], start=True,
                                             stop=True)
                            lexp = att_pool.tile([128, SC], f32, tag="lexpsb")
                            nc.vector.tensor_copy(lexp[:], lexp_ps[:])
                            outn = att_pool.tile([128, SC], f32, tag="outn")
                            nc.vector.tensor_mul(outn[:], pv[i][:], lexp[:])
                            nc.sync.dma_start(
                                outT_d[h * 128:(h + 1) * 128, qsl], outn[:]
                            )

                    pending_norm[0] = norm_group
        flush_norm()

    _dedup_ldweights(nc)
    nc.finalize()
    return nc


def _dedup_ldweights(nc):
    """Remove Ldweights whose weights AP equals the previous load in the
    same block: the PE array still holds those weights, so the paired
    matmul can use them directly. Waits/updates of a removed load move to
    the next PE instruction so the sync protocol is preserved."""
    import concourse.mybir as mb

    removed = 0
    for blk in nc.m.functions[0].blocks:
        last_key = None
        pending_sync = []
        new_insts = []
        for inst in blk.instructions:
            if isinstance(inst, mb.InstLdweights):
                key = str(inst.ins[0])
                if key == last_key:
                    if inst.sync_info is not None:
                        pending_sync.append(inst.sync_info)
                    removed += 1
                    continue
                last_key = key
            if pending_sync and getattr(inst, "engine", None) == mb.EngineType.PE:
                si = inst.sync_info
                if si is None:
                    si = mb.SyncInfo(on_wait=[], on_update=[])
                    inst.sync_info = si
                for ps in pending_sync:
                    si.on_wait.extend(ps.on_wait)
                    si.on_update.extend(ps.on_update)
                pending_sync = []
            new_insts.append(inst)
        assert not pending_sync, "dangling sync from removed Ldweights"
        blk.instructions[:] = new_insts
    return removed


def _get_runner(loop_n: int = 1):
    """Build nc once and a cached jitted shard_map callable (axon/PJRT)."""
    key = f"runner{loop_n}"
    if key in _CACHE:
        return _CACHE[key]

    import jax
    import jax.numpy as jnp  # noqa: F401
    from jax.sharding import Mesh, PartitionSpec
    from jax.experimental.shard_map import shard_map
    from concourse.bass2jax import (
        install_neuronx_cc_hook, _bass_exec_p, partition_id_tensor,
    )
    import concourse.mybir as _mybir

    nc = _build_nc(loop_n)
    install_neuronx_cc_hook()

    partition_name = nc.partition_id_tensor.name if nc.partition_id_tensor else None
    in_names, out_names, out_avals, zero_outs = [], [], [], []
    for alloc in nc.m.functions[0].allocations:
        if not isinstance(alloc, _mybir.MemoryLocationSet):
            continue
        name = alloc.memorylocations[0].name
        if alloc.kind == "ExternalInput":
            if name != partition_name:
                in_names.append(name)
        elif alloc.kind == "ExternalOutput":
            shape = tuple(alloc.tensor_shape)
            npdt = _mybir.dt.np(alloc.dtype)
            out_avals.append(jax.core.ShapedArray(shape, npdt))
            out_names.append(name)
            zero_outs.append(np.zeros(shape, npdt))

    n_params = len(in_names)
    n_outs = len(out_avals)
    all_in_names = in_names + out_names
    if partition_name is not None:
        all_in_names.append(partition_name)
    donate = tuple(range(n_params, n_params + n_outs))

    def _body(*args):
        operands = list(args)
        if partition_name is not None:
            operands.append(partition_id_tensor())
        outs = _bass_exec_p.bind(
            *operands,
            out_avals=tuple(out_avals),
            in_names=tuple(all_in_names),
            out_names=tuple(out_names),
            lowering_input_output_aliases=(),
            sim_require_finite=True,
            sim_require_nnan=True,
            nc=nc,
        )
        return tuple(outs)

    devices = jax.devices()[:N_CORES]
    mesh = Mesh(np.asarray(devices), ("core",))
    in_specs = (PartitionSpec("core"),) * (n_params + n_outs)
    out_specs = (PartitionSpec("core"),) * n_outs
    fn = jax.jit(
        shard_map(_body, mesh=mesh, in_specs=in_specs, out_specs=out_specs,
                  check_rep=False),
        donate_argnums=donate,
        keep_unused=True,
    )

    runner = (fn, in_names, out_names, out_avals, zero_outs)
    _CACHE[key] = runner
    return runner


def _host_prep(positions, hidden_states, w_qkv):
    """Build the per-core input maps (shard + layout prep, no reference math)."""
    import ml_dtypes
    bf = ml_dtypes.bfloat16

    positions = np.asarray(positions)
    hidden_states = np.ascontiguousarray(np.asarray(hidden_states, dtype=np.float32))
    w_qkv = np.ascontiguousarray(np.asarray(w_qkv, dtype=np.float32))

    half = D // 2
    inv_freq = 1.0 / (ROPE_BASE ** (np.arange(half, dtype=np.float32) / half))
    ang = positions.astype(np.float32)[:, None] * inv_freq[None, :]  # [S, 64]
    cos = np.cos(ang).astype(np.float32)  # [S, 64]
    sin = np.sin(ang).astype(np.float32)
    cosF = np.empty((128, S), np.float32)
    sinS = np.empty((128, S), np.float32)
    cosF[:half] = cos.T
    cosF[half:] = cos.T
    sinS[:half] = -sin.T
    sinS[half:] = sin.T
    cosF = cosF.astype(bf)
    sinS = sinS.astype(bf)

    ident = np.eye(128, dtype=np.float32).astype(bf)
    perm = np.roll(np.eye(128, dtype=np.float32), 64, axis=0).astype(bf)
    # mask01[oi][m, q] = 1 where query q may see key row m (offset oi*128)
    mask01 = np.zeros((4, 128, SC), np.float32)
    for oi, o in enumerate([0, 128, 256, 384]):
        for m in range(128):
            mask01[oi, m, min(m + o, SC):] = 1.0
    mask01_flat = np.ascontiguousarray(
        mask01.transpose(1, 0, 2).reshape(128, 4 * SC)
    ).astype(bf)
    onesc = np.ones((128, 1), bf)
    onesr = np.ones((128, 128), bf)

    hT = [np.ascontiguousarray(hidden_states[b].T.astype(bf)) for b in range(B)]

    q_sz = NUM_HEADS * D
    in_maps = []
    for c in range(N_CORES):
        b, tp = divmod(c, 4)
        kv = tp // 2
        wq = w_qkv[:, tp * 1024:(tp + 1) * 1024]
        wk = w_qkv[:, q_sz + kv * 128: q_sz + (kv + 1) * 128]
        wv = w_qkv[:, q_sz + NUM_KV_HEADS * D + kv * 128:
                      q_sz + NUM_KV_HEADS * D + (kv + 1) * 128]
        wc = np.concatenate([wq, wk, wv], axis=1)  # [4096, 1280]
        # tile to [n*128+p, k*128+m] = wc[k*128+p, n*128+m]
        wc_t = np.ascontiguousarray(
            wc.reshape(HIDDEN // 128, 128, NC_CHUNKS, 128)
            .transpose(2, 1, 0, 3)
            .reshape(NC_CHUNKS * 128, HIDDEN)
            .astype(bf)
        )
        in_maps.append({
            "hT": hT[b], "wc": wc_t, "cosF": cosF, "sinS": sinS,
            "ident": ident, "perm": perm,
            "mask01": mask01_flat, "onesc": onesc, "onesr": onesr,
        })
    return in_maps


def run_device(in_maps):
    """Run the compiled kernel on 8 cores; returns list of per-core outputs."""
    fn, in_names, out_names, out_avals, zero_outs = _get_runner()
    per_core = [[np.asarray(m[nm]) for nm in in_names] for m in in_maps]
    concat_in = [
        np.concatenate([per_core[c][i] for c in range(N_CORES)], axis=0)
        for i in range(len(in_names))
    ]
    concat_zeros = [
        np.zeros((N_CORES * z.shape[0], *z.shape[1:]), z.dtype) for z in zero_outs
    ]
    out_arrs = fn(*concat_in, *concat_zeros)
    return [
        {
            nm: np.asarray(out_arrs[i]).reshape(N_CORES, *out_avals[i].shape)[c]
            for i, nm in enumerate(out_names)
        }
        for c in range(N_CORES)
    ]


def kernel(positions, hidden_states, w_qkv):
    in_maps = _host_prep(positions, hidden_states, w_qkv)
    results = run_device(in_maps)
    out = np.empty((B, S, NUM_HEADS * D), np.float32)
    for c in range(N_CORES):
        b, tp = divmod(c, 4)
        oT = results[c]["outT"].reshape(HEADS_PER_CORE, 128, S)
        out[b, :, tp * 1024:(tp + 1) * 1024] = (
            oT.transpose(2, 0, 1).reshape(S, HEADS_PER_CORE * 128)
        )
    return out
